# revision 11
# baseline (speedup 1.0000x reference)
"""Trainium2 Bass kernel for nn_ChebyshevEncoder.

Math (reference):
  xs = x * scale                                  [b, i]
  T_m = Chebyshev polynomials of xs, m = 0..7     [b, i, m]
  S[b,h,i,k] = sum_m T_m(xs[b,i]) * W[h,i,m,k],   W = kernels * poly  (folded on host)
  S = silu(S); flat to [b, f] with f = (h,i,k);  out = LayerNorm(flat) * gamma + beta

Device strategy (pure data parallel over batch, 8 cores, 512 rows each):
  - Monomial basis B = [x, x^2, ..., x^7, 1] computed with 7 chained DVE
    tensor_tensor multiplies in bf16 (2x perf mode); the exact T->B change
    of basis (cheb2poly) is folded into the weights on the host (f64).
  - Basis tiles transposed to (m, i16)-on-partitions layout by ONE DMA XBAR
    transpose instruction per batch tile (SBUF->SBUF, 2-byte dtype) - no
    TensorE transposes, no casts, no PSUM bounce.
  - Per-16-feature-chunk block-diagonal matmul (K = 8 basis fns x 16
    features = 128) into [128, 2048] 4-bank PSUM tiles, double buffered.
  - ScalarE drains 2048-wide Silu + per-row accumulation (running sum for
    the LayerNorm mean).
  - Sum of squares: heads 0-1 via DVE tensor_tensor square (2x) +
    tensor_scalar reduce (4x); heads 2-3 via Pool-engine STT+accum, so the
    three compute engines share the post-silu passes.
  - Quake rsqrt + 2 Newton steps on DVE; normalize via one 4x tensor_scalar
    with per-partition scale/bias; output written bf16, upcast on host.
"""

import os

import numpy as np
import ml_dtypes

BATCH = 4096
INPUT = 512
MAX_TERMS = 8
HEADS = 4
KSIZE = 8
F = HEADS * INPUT * KSIZE  # 16384
N_CORES = 8
ROWS = BATCH // N_CORES    # 512 rows per core
P = 128
NBT = ROWS // P            # 4 batch tiles per core
NCHUNK = INPUT // 16       # 32 feature chunks (16 features each)
LN_EPS = 1e-5

# basis slot order in the tt buffer: [x, x^2, ..., x^7, ones]
_BMAP = [1, 2, 3, 4, 5, 6, 7, 0]

# B basis in monomial coeffs (index = degree): pure monomials
_BPOLYS = [[1 if d == p else 0 for d in range(p + 1)] for p in range(8)]

_NC_CACHE = {}
_LAST_EXEC_NS = {}
_ACT_FN = "Silu"  # debug knob: CoreSim may lack Silu; tests may set "Sigmoid"


def _cheb_to_b_matrix():
    """C with T_m = sum_mp C[m, mp] * B_mp (exact, small ints)."""
    bmat = np.zeros((8, 8))
    for i, p in enumerate(_BPOLYS):
        bmat[i, : len(p)] = p
    tmat = np.zeros((8, 8))
    for m in range(8):
        c = np.zeros(8)
        c[m] = 1
        tmat[m, : m + 1] = np.polynomial.chebyshev.cheb2poly(c)
    C = np.linalg.solve(bmat.T, tmat.T).T
    assert np.abs(C @ bmat - tmat).max() < 1e-9
    return C


def _build_weights(poly_weights, kernels):
    """Fold poly into kernels, change basis, lay out as [chunk, K=128, N=512] bf16.

    K rows: m_blk*16 + i16 with basis order _BMAP; N cols: h*128 + i16*8 + k.
    """
    W = kernels.astype(np.float64) * poly_weights.astype(np.float64)[:, :, None, :]
    C = _cheb_to_b_matrix()
    WB = np.einsum("himk,mn->nhik", W, C)          # [8(mp), H, I, K]
    WBr = WB[_BMAP].reshape(8, HEADS, NCHUNK, 16, KSIZE)  # [m_blk, h, c, i16, k]
    Wdev = np.zeros((NCHUNK, 8, 16, HEADS, 16, KSIZE), np.float64)
    ii = np.arange(16)
    # Wdev[c, m_blk, i, h, i, k] = WBr[m_blk, h, c, i, k]
    # advanced indices (positions 2 and 4) land in front: LHS view is [16, c, 8, h, k]
    Wdev[:, :, ii, :, ii, :] = np.transpose(WBr, (3, 2, 0, 1, 4))
    Wdev = Wdev.reshape(NCHUNK, 128, 512)
    # SBUF layout: [partition K=128, chunk-major free] so the DMA is contiguous
    Wdev = np.ascontiguousarray(Wdev.transpose(1, 0, 2).reshape(128, NCHUNK * 512))
    return Wdev.astype(ml_dtypes.bfloat16)


def _build_nc(apply_gamma, apply_beta):
    from concourse import bacc
    import concourse.mybir as mybir
    from concourse.tile import TileContext

    dt = mybir.dt
    AF = mybir.ActivationFunctionType
    OP = mybir.AluOpType

    nc = bacc.Bacc(None, target_bir_lowering=False)

    x_d = nc.dram_tensor("x", [ROWS, INPUT], dt.float32, kind="ExternalInput")
    w_d = nc.dram_tensor("wb", [P, NCHUNK * 512], dt.bfloat16, kind="ExternalInput")
    sc_d = nc.dram_tensor("scale_bc", [P, INPUT], dt.float32, kind="ExternalInput")
    g_d = b_d = None
    if apply_gamma:
        g_d = nc.dram_tensor("gamma_bc", [P, F], dt.bfloat16, kind="ExternalInput")
    if apply_beta:
        b_d = nc.dram_tensor("beta_bc", [P, F], dt.bfloat16, kind="ExternalInput")
    y_d = nc.dram_tensor("y", [ROWS, F], dt.bfloat16, kind="ExternalOutput")

    with TileContext(nc) as tc:
        with (
            tc.tile_pool(name="const", bufs=1) as constp,
            tc.tile_pool(name="xin", bufs=1) as xinp,
            tc.tile_pool(name="tt", bufs=2) as ttp,
            tc.tile_pool(name="tt32", bufs=1) as tt32p,
            tc.tile_pool(name="lq", bufs=NBT) as lqp,
            tc.tile_pool(name="sbig", bufs=2) as sp,
            tc.tile_pool(name="sqd", bufs=2) as sqdp,
            tc.tile_pool(name="stats", bufs=2) as stp,
            tc.tile_pool(name="mm", bufs=2, space="PSUM") as mmp,
        ):
            x_sb = xinp.tile([P, NBT * INPUT], dt.float32)
            nc.sync.dma_start(
                out=x_sb.rearrange("p (t i) -> p t i", t=NBT),
                in_=x_d.rearrange("(t p) i -> p t i", p=P),
            )
            sc_sb = constp.tile([P, INPUT], dt.float32)
            nc.sync.dma_start(out=sc_sb[:], in_=sc_d[:])
            # weights in 8 sub-loads so the first matmuls start early
            w_sb = constp.tile([P, NCHUNK * 512], dt.bfloat16)
            for wc in range(8):
                sl = slice(wc * 2048, (wc + 1) * 2048)
                nc.scalar.dma_start(out=w_sb[:, sl], in_=w_d[:, sl])
            if apply_gamma:
                g_sb = constp.tile([P, F], dt.bfloat16)
                nc.scalar.dma_start(out=g_sb[:], in_=g_d[:])
            if apply_beta:
                b_sb = constp.tile([P, F], dt.bfloat16)
                nc.scalar.dma_start(out=b_sb[:], in_=b_d[:])

            # quake-rsqrt integer constants
            magic = constp.tile([P, 1], dt.int32)
            nc.vector.memset(magic[:], 0x5F3759DF)
            shift1 = constp.tile([P, 1], dt.int32)
            nc.vector.memset(shift1[:], 1)

            junk = constp.tile([P, 4096], dt.bfloat16)  # ACT square-pass sink

            v = nc.vector
            a = nc.scalar
            act_fn = getattr(AF, _ACT_FN)

            # ---- basis + transpose for all tiles upfront so the PE never
            # waits on the DVE queue; tile 0 on DVE (fast start), rest on the
            # idle Pool engine ----
            lqs = []
            for bt in range(NBT):
                e = v if bt == 0 else nc.gpsimd
                xt = x_sb[:, bt * INPUT : (bt + 1) * INPUT]
                tt = ttp.tile([P, 8 * INPUT], dt.bfloat16)
                # layout: col = c*128 + s*16 + i16, slot s holds basis _BMAP[s]
                t4 = tt.rearrange("p (c m i) -> p c m i", c=NCHUNK, m=8)
                x3 = xt.rearrange("p (c i) -> p c i", c=NCHUNK)
                s3 = sc_sb.rearrange("p (c i) -> p c i", c=NCHUNK)
                # chain in f32 (compounded bf16 rounding is the top error
                # source), then one cast to bf16
                tt32 = tt32p.tile([P, 7 * INPUT], dt.float32)
                f4 = tt32.rearrange("p (c m i) -> p c m i", c=NCHUNK, m=7)
                e.tensor_tensor(f4[:, :, 0], x3, s3, OP.mult)                    # x
                e.tensor_tensor(f4[:, :, 1], f4[:, :, 0], f4[:, :, 0], OP.mult)  # x^2
                e.tensor_tensor(f4[:, :, 2], f4[:, :, 0], f4[:, :, 1], OP.mult)  # x^3
                e.tensor_tensor(f4[:, :, 3], f4[:, :, 1], f4[:, :, 1], OP.mult)  # x^4
                e.tensor_tensor(f4[:, :, 4], f4[:, :, 1], f4[:, :, 2], OP.mult)  # x^5
                e.tensor_tensor(f4[:, :, 5], f4[:, :, 2], f4[:, :, 2], OP.mult)  # x^6
                e.tensor_tensor(f4[:, :, 6], f4[:, :, 2], f4[:, :, 3], OP.mult)  # x^7
                e.tensor_copy(t4[:, :, 0:7, :], f4[:, :, :, :])                  # -> bf16
                e.memset(t4[:, :, 7], 1.0)                                       # ones
                # one blocked XBAR transpose: lq[p, c, b] = tt[b, c*128+p]
                lq = lqp.tile([P, 8 * INPUT], dt.bfloat16)
                nc.sync.dma_start_transpose(
                    lq.rearrange("p (c b) -> p c b", c=NCHUNK), tt[:]
                )
                lqs.append(lq)

            for bt in range(NBT):
                lq = lqs[bt]
                s_t = sp.tile([P, F], dt.bfloat16)
                # f = h*4096 + (ig*4 + j)*128 + i16*8 + k
                s6 = s_t.rearrange(
                    "p (h g j i k) -> p h g j i k", h=HEADS, g=8, j=4, i=16
                )
                strip = stp.tile([P, 17], dt.float32, tag="strip")

                for ig in range(8):
                    mm = mmp.tile([P, 2048], dt.float32, space="PSUM", tag="mm")
                    for j in range(4):
                        c = 4 * ig + j
                        nc.tensor.matmul(
                            mm[:, j * 512 : (j + 1) * 512],
                            lq[:, c * P : (c + 1) * P],
                            w_sb[:, c * 512 : (c + 1) * 512],
                            start=True,
                            stop=True,
                        )
                    # silu drain with running row-sum (LN mean)
                    a.activation(
                        s6[:, :, ig, :, :, :],
                        mm.rearrange("p (j h i k) -> p h j i k", j=4, h=HEADS, i=16),
                        act_fn,
                        accum_out=strip[:, ig : ig + 1],
                    )
                    # sum-of-squares, chunks j=0..2, on DVE (1x STT+accum)
                    sqa = sqdp.tile([P, 1536], dt.bfloat16)
                    sv = s6[:, :, ig, 0:3, :, :]
                    v.scalar_tensor_tensor(
                        sqa.rearrange("p (h j i k) -> p h j i k", h=HEADS, j=3, i=16),
                        sv,
                        1.0,
                        sv,
                        OP.mult,
                        OP.mult,
                        accum_out=strip[:, 8 + ig : 9 + ig],
                    )
                # sum-of-squares for chunk j=3 of all igs on ACT (Square table
                # lives in the same act set as Silu - no table switch)
                sv3 = s6[:, :, :, 3, :, :]
                a.activation(
                    junk.rearrange("p (h g i k) -> p h g i k", h=HEADS, g=8, i=16),
                    sv3,
                    AF.Square,
                    accum_out=strip[:, 16:17],
                )

                # ---- layernorm stats ----
                st = stp.tile([P, 16], dt.float32, tag="st")
                sti = st.bitcast(dt.int32)
                rowsum = st[:, 1:2]
                v.tensor_reduce(rowsum, strip[:, 0:8], mybir.AxisListType.X, OP.add)
                sumsq = st[:, 3:4]
                v.tensor_reduce(sumsq, strip[:, 8:17], mybir.AxisListType.X, OP.add)
                mean = st[:, 4:5]
                v.tensor_scalar(mean, rowsum, 1.0 / F, None, OP.mult)
                ex2 = st[:, 5:6]
                v.tensor_scalar(ex2, sumsq, 1.0 / F, None, OP.mult)
                nm2 = st[:, 6:7]
                v.tensor_scalar(nm2, mean, mean, -1.0, OP.mult, OP.mult)
                vpe = st[:, 7:8]
                v.scalar_tensor_tensor(vpe, ex2, LN_EPS, nm2, OP.add, OP.add)
                # quake rsqrt + 2 Newton steps (all DVE; keeps ACT table on Silu)
                bits = sti[:, 8:9]
                v.tensor_scalar(bits, sti[:, 7:8], shift1[:, 0:1], None, OP.arith_shift_right)
                r0i = sti[:, 9:10]
                v.tensor_tensor(r0i, magic[:, 0:1], bits, OP.subtract)
                r = st[:, 9:10]  # same bytes as r0i, viewed f32
                for it in range(2):
                    m1 = st[:, 10 + 2 * it : 11 + 2 * it]
                    v.tensor_tensor(m1, r, r, OP.mult)
                    m2 = st[:, 11 + 2 * it : 12 + 2 * it]
                    v.tensor_tensor(m2, m1, vpe, OP.mult)
                    v.tensor_scalar(m2, m2, -0.5, 1.5, OP.mult, OP.add)
                    rn = st[:, 14:15] if it == 1 else st[:, 10 + 2 * it : 11 + 2 * it]
                    v.tensor_tensor(rn, r, m2, OP.mult)
                    r = rn
                rstd = r
                biasp = st[:, 15:16]
                v.tensor_scalar(biasp, mean, rstd, -1.0, OP.mult, OP.mult)

                # ---- normalize (+ gamma/beta) in place, 4x DVE; split in two
                # halves so the output DMA starts before the whole tile is done
                for hf in range(2):
                    sl = s_t[:, hf * (F // 2) : (hf + 1) * (F // 2)]
                    v.tensor_scalar(sl, sl, rstd, biasp, OP.mult, OP.add)
                    if apply_gamma:
                        v.tensor_tensor(
                            sl, sl, g_sb[:, hf * (F // 2) : (hf + 1) * (F // 2)], OP.mult
                        )
                    if apply_beta:
                        v.tensor_tensor(
                            sl, sl, b_sb[:, hf * (F // 2) : (hf + 1) * (F // 2)], OP.add
                        )
                    nc.sync.dma_start(
                        out=y_d[bt * P : (bt + 1) * P, hf * (F // 2) : (hf + 1) * (F // 2)],
                        in_=sl,
                    )

    nc.compile()
    return nc


def _get_nc(apply_gamma, apply_beta):
    key = (apply_gamma, apply_beta)
    if key not in _NC_CACHE:
        _NC_CACHE[key] = _build_nc(apply_gamma, apply_beta)
    return _NC_CACHE[key]


def _install_axon_ntff_hook():
    """Benchmark-only: provide antenv.axon_hooks if the image lacks it, so
    run_bass_kernel_spmd(trace=True) can capture NTFF profiles under axon."""
    import sys
    import types
    import ctypes
    import contextlib

    try:
        from antenv.axon_hooks import get_axon_ntff_profile_hook  # noqa: F401

        return
    except ImportError:
        pass
    so_path = os.environ.get("PJRT_LIBRARY_PATH", "/opt/axon/libaxon_pjrt.so")
    try:
        lib = ctypes.CDLL(so_path)
    except OSError:
        return
    if not hasattr(lib, "axon_start_nrt_profile"):
        return
    lib.axon_start_nrt_profile.argtypes = [
        ctypes.POINTER(ctypes.c_int64),
        ctypes.c_size_t,
    ]
    lib.axon_start_nrt_profile.restype = ctypes.c_int64
    lib.axon_stop_nrt_profile.argtypes = [ctypes.c_char_p]
    lib.axon_stop_nrt_profile.restype = ctypes.c_int64

    @contextlib.contextmanager
    def _hook(output_dir, device_ids):
        import jax

        jax.devices()
        if device_ids:
            ids = (ctypes.c_int64 * len(device_ids))(*device_ids)
            rc = lib.axon_start_nrt_profile(ids, len(device_ids))
        else:
            rc = lib.axon_start_nrt_profile(None, 0)
        if rc != 0:
            raise RuntimeError(f"axon_start_nrt_profile rc={rc}")
        try:
            yield
        finally:
            n = lib.axon_stop_nrt_profile(str(output_dir).encode())
            print(f"ntff profile: {n} file(s) written to {output_dir}")

    mod = types.ModuleType("antenv.axon_hooks")
    mod.get_axon_ntff_profile_hook = lambda: _hook
    mod.set_axon_ntff_profile_hook = lambda h: None
    sys.modules["antenv.axon_hooks"] = mod
    import antenv

    antenv.axon_hooks = mod


def kernel(x, scale_param, poly_weights, kernels, ln_gamma, ln_beta):
    from concourse.bass_utils import run_bass_kernel_spmd

    x = np.asarray(x, dtype=np.float32)
    scale_param = np.asarray(scale_param, dtype=np.float32)
    poly_weights = np.asarray(poly_weights, dtype=np.float32)
    kernels = np.asarray(kernels, dtype=np.float32)
    ln_gamma = np.asarray(ln_gamma, dtype=np.float32)
    ln_beta = np.asarray(ln_beta, dtype=np.float32)

    apply_gamma = not np.all(ln_gamma == 1.0)
    apply_beta = not np.all(ln_beta == 0.0)

    wdev = _build_weights(poly_weights, kernels)
    sc_bc = np.ascontiguousarray(np.broadcast_to(scale_param[None, :], (P, INPUT))).astype(
        np.float32
    )

    base = {
        "wb": wdev,
        "scale_bc": sc_bc,
    }
    if apply_gamma:
        base["gamma_bc"] = np.ascontiguousarray(
            np.broadcast_to(ln_gamma[None, :], (P, F))
        ).astype(ml_dtypes.bfloat16)
    if apply_beta:
        base["beta_bc"] = np.ascontiguousarray(
            np.broadcast_to(ln_beta[None, :], (P, F))
        ).astype(ml_dtypes.bfloat16)

    in_maps = []
    for core in range(N_CORES):
        m = dict(base)
        m["x"] = np.ascontiguousarray(x[core * ROWS : (core + 1) * ROWS])
        in_maps.append(m)

    nc = _get_nc(apply_gamma, apply_beta)

    trace = os.environ.get("KBENCH_TRACE", "0") == "1"
    if trace:
        _install_axon_ntff_hook()
    res = run_bass_kernel_spmd(
        nc,
        in_maps,
        core_ids=list(range(N_CORES)),
        trace=trace,
    )
    _LAST_EXEC_NS["exec_time_ns"] = res.exec_time_ns
    _LAST_EXEC_NS["trace"] = res.instructions_and_trace[1] if res.instructions_and_trace else None

    out = np.concatenate([r["y"] for r in res.results], axis=0)
    return out.astype(np.float32)
